# revision 15
# baseline (speedup 1.0000x reference)
"""Trainium2 Bass kernel for a diagonal-A linear dynamical system (LDS).

    Bu = inputs @ B            [B, T, S]
    h_t = h_{t-1} * A + Bu_t   (scan over T, diagonal A)
    y_t = h_t @ C              [B, T, O]

Shapes: inputs [16, 4096, 256], A [256], B [256, 256], C [256, 256],
h0 [256]; all float32.

Sharding: data-parallel over batch across 8 NeuronCores (2 batches per
core); A/B/C/h0 replicated.

Radix-2 decimated dataflow: the serial scan (DVE ~2.2 ns/col, no
faster engine exists) only runs over ODD timesteps, halving its
columns; even-timestep outputs are reconstructed algebraically on the
PE without ever materializing even states:

    c_k     = u_{2k} @ (B diag A) + u_{2k+1} @ B      (PE, PSUM)
    hodd_k  = hodd_{k-1} * A^2 + c_k                  (DVE scan, T/2)
    y_{2k+1}= hodd_k @ C                              (PE)
    y_{2k}  = hodd_{k-1} @ (diag(A) C) + u_{2k} @ (B C)   (PE)

Host-side prep: u is parity-split/transposed to [b, par, ih, i, t2]
bf16 (no strided PE reads, no on-chip transposes); B, B diag A, BC are
bf16, C and diag(A)C are f32r; output leaves as bf16 [b, par, oh, o,
t2] and the host reassembles/upcasts. (bf16 path rel err ~3e-3, gate
2e-2.) hodd keeps a leading h0 column so the shifted operand for
y_even is a plain contiguous slice.

Engines: u loads on Sync, y stores + C/AC/h0 consts on gpsimd SWDGE,
A2/B/BA/M split over ACT+Sync at startup, scans on DVE only, PSUM->
SBUF copies on ACT. First supertile is processed in 256-col pieces so
the first scan starts after only 256 KB of input.
"""

import ml_dtypes
import numpy as np

import concourse.bacc as bacc
import concourse.bass as bass
import concourse.mybir as mybir
import concourse.tile as tile
from concourse import bass_utils

BATCH, T, D = 16, 4096, 256
NCORES = 8
BLOC = BATCH // NCORES  # batches per core
T2 = T // 2             # decimated time
TC = 512                # t2 supertile (scan granularity)
NJ = T2 // TC           # supertiles per sequence
HC = 256                # first-supertile piece size (t2 cols)
F32 = mybir.dt.float32
F32R = mybir.dt.float32r
BF16 = mybir.dt.bfloat16

_CACHE: dict = {}


def _build_nc():
    nc = bacc.Bacc(trn_type="TRN2", target_bir_lowering=False)

    # u parity-split + transposed on host: [b, par, ihalf, i, t2] bf16
    u = nc.dram_tensor("u", [BLOC, 2, 2, 128, T2], BF16, kind="ExternalInput")
    A2d = nc.dram_tensor("A2", [128, 2], F32, kind="ExternalInput")      # A^2
    Bd = nc.dram_tensor("B", [2, 128, D], BF16, kind="ExternalInput")    # [ih, i, s]
    BAd = nc.dram_tensor("BA", [2, 128, D], BF16, kind="ExternalInput")  # B diag(A)
    Md = nc.dram_tensor("M", [2, 128, D], BF16, kind="ExternalInput")    # B C
    Cd = nc.dram_tensor("C", [2, 128, D], F32R, kind="ExternalInput")    # [kh, s, o]
    ACd = nc.dram_tensor("AC", [2, 128, D], F32R, kind="ExternalInput")  # diag(A) C
    h0d = nc.dram_tensor("h0", [128, 2], F32, kind="ExternalInput")
    # y out [b, par, ohalf, o, t2] bf16; host reassembles + upcasts
    y = nc.dram_tensor("y", [BLOC, 2, 2, 128, T2], BF16, kind="ExternalOutput")

    u_r = u[:].rearrange("b par ih i t -> b i par ih t")
    y_r = y[:].rearrange("b par oh o t -> b o par oh t")
    B_r = Bd[:].rearrange("ih i s -> i ih s")
    BA_r = BAd[:].rearrange("ih i s -> i ih s")
    M_r = Md[:].rearrange("ih i s -> i ih s")
    C_r = Cd[:].rearrange("k s o -> s k o")
    AC_r = ACd[:].rearrange("k s o -> s k o")

    mult = mybir.AluOpType.mult
    add = mybir.AluOpType.add

    with tile.TileContext(nc) as tc:
        with (
            tc.tile_pool(name="const", bufs=1) as const,
            tc.tile_pool(name="usb", bufs=3) as usb,
            tc.tile_pool(name="ysb", bufs=3) as ysb,
            tc.tile_pool(name="hpool", bufs=1) as hpool,
            tc.tile_pool(name="ps_c", bufs=3, space="PSUM") as ps_c,
            tc.tile_pool(name="ps_y", bufs=4, space="PSUM") as ps_y,
        ):
            u_tiles = {}

            def load_u_piece(b, piece):
                # first-supertile piece: [128, par, ih, HC]
                u_t = usb.tile([128, 2, 2, HC], BF16, tag="u_t", bufs=4, name="u_t")
                t0 = piece * HC
                nc.sync.dma_start(u_t, u_r[b, :, :, :, t0 : t0 + HC])
                u_tiles[(b, 0, piece)] = u_t

            def load_u_full(b, j):
                u_t = usb.tile([128, 2, 2, TC], BF16, tag="u_f", bufs=3, name="u_f")
                nc.sync.dma_start(u_t, u_r[b, :, :, :, j * TC : (j + 1) * TC])
                u_tiles[(b, j, 0)] = u_t

            # startup: A2 on ACT (feeds DVE A2_bc), BA+B on Sync before u
            # (they gate the first c matmuls), M/C/AC/h0 on gpsimd SWDGE.
            A2_col = const.tile([128, 2], F32, name="A2_col")
            nc.scalar.dma_start(A2_col, A2d[:])
            BA_sb = const.tile([128, 2, D], BF16, name="BA_sb")
            nc.sync.dma_start(BA_sb, BA_r)
            B_sb = const.tile([128, 2, D], BF16, name="B_sb")
            nc.sync.dma_start(B_sb, B_r)
            load_u_piece(0, 0)
            load_u_piece(0, 1)
            load_u_piece(1, 0)
            load_u_piece(1, 1)

            M_sb = const.tile([128, 2, D], BF16, name="M_sb")
            nc.gpsimd.dma_start(M_sb, M_r)
            C_sb = const.tile([128, 2, D], F32R, name="C_sb")
            nc.gpsimd.dma_start(C_sb, C_r)
            AC_sb = const.tile([128, 2, D], F32R, name="AC_sb")
            nc.gpsimd.dma_start(AC_sb, AC_r)
            h0c = const.tile([128, 2], F32, name="h0c")
            nc.gpsimd.dma_start(h0c, h0d[:])

            ones = const.tile([128, TC], F32, name="ones")
            nc.vector.memset(ones, 1.0)
            A2_bc = const.tile([128, 2, TC], F32, name="A2_bc")
            for m in range(2):
                nc.vector.tensor_scalar_mul(A2_bc[:, m], ones, A2_col[:, m : m + 1])

            # hodd states [128s, b, mhalf, 1 + T2] (leading col = h0)
            hod = hpool.tile([128, BLOC, 2, 1 + T2], F32R, name="hod")
            for b in range(BLOC):
                for m in range(2):
                    nc.scalar.copy(hod[:, b, m, 0:1], h0c[:, m : m + 1])

            def c_mms(c_ps, u_t, m, cols):
                # c = u_even @ BA + u_odd @ B, contraction over i halves
                first = True
                for var_sb, par in ((BA_sb, 0), (B_sb, 1)):
                    for ih in range(2):
                        nc.tensor.matmul(
                            c_ps[:, :cols],
                            var_sb[:, ih, m * 128 : (m + 1) * 128],
                            u_t[:, par, ih],
                            start=first,
                            stop=(par == 1 and ih == 1),
                        )
                        first = False

            def scan(b, m, t2a, cols, c_ps, first_chunk):
                init = (
                    h0c[:, m : m + 1]
                    if first_chunk
                    else hod[:, b, m, t2a : t2a + 1]
                )
                nc.vector.tensor_tensor_scan(
                    hod[:, b, m, 1 + t2a : 1 + t2a + cols],
                    A2_bc[:, m, :cols],
                    c_ps[:, :cols],
                    init,
                    op0=mult,
                    op1=add,
                )

            def c_scan(b, j):
                if j == 0:
                    for piece in range(2):
                        u_t = u_tiles[(b, 0, piece)]
                        for m in range(2):
                            c_ps = ps_c.tile([128, TC], F32, tag="c", name="c_ps")
                            c_mms(c_ps, u_t, m, HC)
                            scan(b, m, piece * HC, HC, c_ps, piece == 0)
                else:
                    u_t = u_tiles[(b, j, 0)]
                    for m in range(2):
                        c_ps = ps_c.tile([128, TC], F32, tag="c", name="c_ps")
                        c_mms(c_ps, u_t, m, TC)
                        scan(b, m, j * TC, TC, c_ps, False)

            def y_stage(b, j):
                t2a = j * TC
                y_sb = ysb.tile([128, 2, 2, TC], BF16, tag="y_sb", name="y_sb")
                for oh in range(2):
                    # odd outputs: y_{2k+1} = hodd_k @ C
                    yo = ps_y.tile([128, TC], F32, tag="y", name="y_odd")
                    for kh in range(2):
                        nc.tensor.matmul(
                            yo,
                            C_sb[:, kh, oh * 128 : (oh + 1) * 128],
                            hod[:, b, kh, 1 + t2a : 1 + t2a + TC],
                            start=(kh == 0),
                            stop=(kh == 1),
                        )
                    nc.scalar.copy(y_sb[:, 1, oh], yo)
                    # even outputs: y_{2k} = hodd_{k-1} @ AC + u_even @ M
                    ye = ps_y.tile([128, TC], F32, tag="y", name="y_even")
                    for kh in range(2):
                        nc.tensor.matmul(
                            ye,
                            AC_sb[:, kh, oh * 128 : (oh + 1) * 128],
                            hod[:, b, kh, t2a : t2a + TC],
                            start=(kh == 0),
                            stop=False,
                            skip_group_check=True,
                        )
                    if j == 0:
                        for piece in range(2):
                            u_t = u_tiles[(b, 0, piece)]
                            for ih in range(2):
                                nc.tensor.matmul(
                                    ye[:, piece * HC : (piece + 1) * HC],
                                    M_sb[:, ih, oh * 128 : (oh + 1) * 128],
                                    u_t[:, 0, ih],
                                    start=False,
                                    stop=(piece == 1 and ih == 1),
                                    skip_group_check=True,
                                )
                    else:
                        u_t = u_tiles[(b, j, 0)]
                        for ih in range(2):
                            nc.tensor.matmul(
                                ye,
                                M_sb[:, ih, oh * 128 : (oh + 1) * 128],
                                u_t[:, 0, ih],
                                start=False,
                                stop=(ih == 1),
                                skip_group_check=True,
                            )
                    nc.scalar.copy(y_sb[:, 0, oh], ye)
                eng = nc.sync if j == NJ - 1 else nc.gpsimd
                eng.dma_start(y_r[b, :, :, :, t2a : t2a + TC], y_sb)

            # software pipeline: y-stage runs one supertile behind c/scan
            for j in range(NJ + 1):
                for b in range(BLOC):
                    if j < NJ:
                        c_scan(b, j)
                        if j + 1 < NJ:
                            load_u_full(b, j + 1)
                    if j >= 1:
                        y_stage(b, j - 1)

    nc.compile()
    return nc


def _get_nc():
    if "nc" not in _CACHE:
        _CACHE["nc"] = _build_nc()
    return _CACHE["nc"]


def make_in_maps(inputs, A, B, C, h0):
    bf16 = ml_dtypes.bfloat16
    u = np.asarray(inputs, dtype=np.float32)
    A = np.asarray(A, np.float32)
    B = np.asarray(B, np.float32)
    C = np.asarray(C, np.float32)
    A2 = np.ascontiguousarray((A * A).reshape(2, 128).T)
    h02 = np.ascontiguousarray(np.asarray(h0, np.float32).reshape(2, 128).T)
    as_w = lambda X, dt: np.ascontiguousarray(X.astype(dt).reshape(2, 128, D))
    Br = as_w(B, bf16)
    BAr = as_w(B * A[None, :], bf16)
    Mr = as_w(B.astype(np.float64) @ C.astype(np.float64), bf16)
    Cr = as_w(C, np.float32)
    ACr = as_w(C * A[:, None], np.float32)
    maps = []
    for c in range(NCORES):
        uc = u[c * BLOC : (c + 1) * BLOC]                    # [BLOC, T, 256]
        up = np.stack([uc[:, 0::2, :], uc[:, 1::2, :]], 1)   # [BLOC, par, T2, 256]
        up = up.transpose(0, 1, 3, 2)                        # [BLOC, par, 256, T2]
        up = np.ascontiguousarray(
            up.reshape(BLOC, 2, 2, 128, T2).astype(bf16)
        )
        maps.append(
            {"u": up, "A2": A2, "B": Br, "BA": BAr, "M": Mr, "C": Cr,
             "AC": ACr, "h0": h02}
        )
    return maps


def kernel(inputs, A, B, C, h0, _trace=False):
    nc = _get_nc()
    in_maps = make_in_maps(inputs, A, B, C, h0)
    res = bass_utils.run_bass_kernel_spmd(
        nc, in_maps, core_ids=list(range(NCORES)), trace=_trace
    )
    outs = []
    for r in res.results:
        yp = np.asarray(r["y"]).astype(np.float32)   # [BLOC, par, oh, 128, T2]
        # t = 2*t2 + par, o = oh*128 + o'
        yc = yp.transpose(0, 4, 1, 2, 3).reshape(BLOC, T, D)
        outs.append(yc)
    out = np.ascontiguousarray(np.concatenate(outs, axis=0), dtype=np.float32)
    if _trace:
        _CACHE["last_result"] = res
    return out
